# revision 43
# baseline (speedup 1.0000x reference)
"""3-layer GCN (GCNConv+BN+ReLU x2, GCNConv+log_softmax) on 8 trn2 NeuronCores.

Strategy (v2): aggregate in input space, transform after. Nodes are
in-degree-sorted and dealt round-robin to 8 cores. Tables T_l hold
h_l(n)*dinv[n] in bf16, node-major (T0 = x*dinv precomputed on host, so
layer 1 needs no collective). Per layer, each core runs a few BIG
transpose-mode dma_gather calls (feature-major output, j-major slot packing
per 128-dst tile), a DVE tensor_reduce per tile for the segment sum, one
PE matmul agg'@W' (+ rank-1 bias matmul via 1/dinv row), and one fused ACT
(relu, scale=dinv^2) producing the next table row, written to the shard.
AllGathers are chunked (4 per layer) and dispatched two gather-calls late
so the CC engine overlaps them with remaining gathers; only the small last
chunk is exposed at the layer boundary. Gather indices are int16 signed
offsets around a mid-table BASE (HW sign-extends); every call is tail-padded
with one block of positive zero-row indices so the trailing-negative drop
rule never bites. The runtime is descriptor-generation-bound on GPSIMD, so
everything else is engineered to hide under it.
"""
import numpy as np

N = 50000
E = 800000
D = 128
D_OUT = 40
D_OUT_PAD = 64
BN_EPS = 1e-5
NCORES = 8
SHARD = N // NCORES              # 6250
SR = 6256                        # shard rows (padded)
TBL = SR * NCORES                # 50048
NTILES = (SHARD + 127) // 128    # 49
BASE = 32768
CAP = 3072                       # max real idxs per gather call
# AllGather chunking (in tiles). The chunk whose table range contains BASE
# (rows 32768..) must be dispatched LAST so the gather's dep on it implies
# all earlier chunks completed (CC queue is in-order). Chunk 2 starts at
# table row 8*4096 = 32768 by construction.
CHUNK_TILES = (16, 16, 2, 11, 4)
NCHUNK = len(CHUNK_TILES)
BASE_CHUNK = 2
# tile processing order: BASE_CHUNK's tiles go last, so its AG is the only
# one exposed at the layer boundary (all other chunk AGs overlap gathers)
def _tile_order():
    b = np.cumsum((0,) + CHUNK_TILES)
    chunks = [list(range(b[i], b[i + 1])) for i in range(NCHUNK)]
    order = []
    for i in range(NCHUNK):
        if i != BASE_CHUNK:
            order += chunks[i]
    order += chunks[BASE_CHUNK]
    return order, [set(c) for c in chunks]


def _chunk_layout():
    b = np.cumsum((0,) + CHUNK_TILES)          # tile bounds, b[-1] == 49
    p = [int(min(x * 128, SR)) for x in b]
    p[-1] = SR                                  # last chunk includes pad rows
    L = [p[i + 1] - p[i] for i in range(NCHUNK)]
    tbl_base = np.concatenate([[0], np.cumsum([NCORES * x for x in L])])
    assert tbl_base[BASE_CHUNK] == BASE
    return p, L, tbl_base


def _preprocess(src, dst):
    import os
    global CAP
    CAP = int(os.environ.get("KERNEL_CAP", CAP))
    p, L, tbl_base = _chunk_layout()
    deg = np.bincount(dst, minlength=N).astype(np.float64) + 1.0
    dinv = (1.0 / np.sqrt(deg)).astype(np.float32)
    order = np.argsort(deg, kind="stable")
    core_of = np.empty(N, np.int64)
    pos_of = np.empty(N, np.int64)
    core_of[order] = np.arange(N) % NCORES
    pos_of[order] = np.arange(N) // NCORES

    pb = np.array(p[1:])                       # chunk upper pos bounds
    def tid_cp(c, pos):
        k = np.searchsorted(pb - 1, pos)       # chunk of pos
        k = np.minimum(k, NCHUNK - 1)
        Lk = np.array(L)[k]
        return tbl_base[k] + c * Lk + (pos - np.array(p[:NCHUNK])[k])

    tid = tid_cp(core_of, pos_of)              # node -> table row
    zid = int(tbl_base[NCHUNK - 1] + (NCORES - 1) * L[NCHUNK - 1]
              + (SR - p[NCHUNK - 1] - 1))
    assert zid == TBL - 1

    es = np.concatenate([src, np.arange(N)])   # + self loops
    ed = np.concatenate([dst, np.arange(N)])
    sid_all = tid[es]
    ec = core_of[ed]
    ep = pos_of[ed]

    counts = np.zeros((NCORES, NTILES * 128), np.int64)
    np.add.at(counts, (ec, ep), 1)
    nb = counts.reshape(NCORES, NTILES, 128).max(axis=(0, 2))  # [NTILES]

    # greedy call grouping over the permuted tile order
    order_t, _ = _tile_order()
    calls = []                                 # (tile_list, nidx)
    cur, acc = [], 0
    for t in order_t:
        w = int(128 * nb[t])
        if acc and acc + w > CAP:
            calls.append((cur, acc))
            cur, acc = [], 0
        cur.append(t)
        acc += w
    calls.append((cur, acc))

    Sl = int((128 * nb).sum())
    tile_off = np.concatenate([[0], np.cumsum(128 * nb)])

    idx_wrapped = []
    dinv_t, dinv2_t, invd_row = [], [], []
    shard_nodes = []
    for c in range(NCORES):
        sel = ec == c
        pos = ep[sel]
        s = sid_all[sel]
        o = np.argsort(pos, kind="stable")
        pos, s = pos[o], s[o]
        cnt = np.bincount(pos, minlength=NTILES * 128)
        starts = np.concatenate([[0], np.cumsum(cnt)[:-1]])
        r = np.arange(len(pos)) - starts[pos]
        t_of = pos // 128
        jj = pos % 128
        flat = tile_off[t_of] + r * 128 + jj
        slots = np.full(Sl, zid, np.int64)
        slots[flat] = s
        stream = []
        for (tlist, nidx) in calls:
            for t in tlist:
                stream.append(slots[tile_off[t]:tile_off[t] + 128 * nb[t]])
            stream.append(np.full(128, zid, np.int64))  # positive tail pad
        arr = np.concatenate(stream)
        idx16 = (arr - BASE).astype(np.int16)
        w16 = idx16.reshape(-1, 16).T
        idx_wrapped.append(np.tile(w16, (8, 1)).copy())

        nodes = order[c::NCORES]               # pos-ordered own nodes
        shard_nodes.append(nodes)
        dv = np.ones(NTILES * 128, np.float32)
        dv[:SHARD] = dinv[nodes]
        dinv_t.append(dv.reshape(NTILES, 128).T.copy())
        dinv2_t.append((dv * dv).reshape(NTILES, 128).T.copy())
        invd_row.append((1.0 / dv)[None, :].copy())
    return (dinv, tid, nb, calls, idx_wrapped, dinv_t, dinv2_t, invd_row,
            shard_nodes, p, tbl_base, zid)


def _build(nb, calls, p, tbl_base):
    import os
    import concourse.bass as bass
    import concourse.tile as tile
    from concourse import bacc, mybir
    NO_AG = bool(int(os.environ.get("KERNEL_NO_AG", "0")))
    NLAYERS = int(os.environ.get("KERNEL_NLAYERS", "3"))
    DBG = bool(int(os.environ.get("KERNEL_DBG", "0")))
    NQ = int(os.environ.get("KERNEL_NQ", "4"))
    GBUFS = int(os.environ.get("KERNEL_GBUFS", "4"))
    RDELAY = int(os.environ.get("KERNEL_RDELAY", "0"))

    f32 = mybir.dt.float32
    bf16 = mybir.dt.float16
    i16 = mybir.dt.int16
    AF = mybir.ActivationFunctionType
    NSWQ = int(os.environ.get("KERNEL_NSWQ", "4"))
    nc = bacc.Bacc("TRN2", num_devices=NCORES, debug=False,
                   num_swdge_queues=NSWQ,
                   dynamic_dma_scratch_size=65536 if NSWQ == 1 else 32768)

    SC = sum((nidx + 128) // 16 for (_, nidx) in calls)
    GMAX = max(nidx for (_, nidx) in calls) + 128
    t0_in = nc.dram_tensor("t0", [TBL, D], bf16, kind="ExternalInput")
    idx_in = nc.dram_tensor("idx", [128, SC], i16, kind="ExternalInput")
    w1_in = nc.dram_tensor("w1", [128, D], bf16, kind="ExternalInput")
    w2_in = nc.dram_tensor("w2", [128, D], bf16, kind="ExternalInput")
    w3_in = nc.dram_tensor("w3", [128, D_OUT_PAD], bf16, kind="ExternalInput")
    b1_in = nc.dram_tensor("b1r", [1, D], bf16, kind="ExternalInput")
    b2_in = nc.dram_tensor("b2r", [1, D], bf16, kind="ExternalInput")
    b3_in = nc.dram_tensor("b3r", [1, D_OUT_PAD], bf16, kind="ExternalInput")
    invd_in = nc.dram_tensor("invd", [1, NTILES * 128], bf16,
                             kind="ExternalInput")
    id_in = nc.dram_tensor("ident", [128, 128], bf16, kind="ExternalInput")
    dinv_in = nc.dram_tensor("dinvt", [128, NTILES], f32, kind="ExternalInput")
    dinv2_in = nc.dram_tensor("dinv2t", [128, NTILES], f32,
                              kind="ExternalInput")
    y_out = nc.dram_tensor("y", [SHARD, D_OUT], f32, kind="ExternalOutput")
    if DBG:
        dbg_sh = nc.dram_tensor("dbgsh", [SR, D], bf16, kind="ExternalOutput")
        dbg_tb = nc.dram_tensor("dbgtb", [TBL, D], bf16, kind="ExternalOutput")

    with tile.TileContext(nc) as tc:
        with tc.tile_pool(name="cst", bufs=1) as cst, \
             tc.tile_pool(name="gp", bufs=GBUFS) as gp, \
             tc.tile_pool(name="wrk", bufs=4) as wrk, \
             tc.tile_pool(name="ps", bufs=2, space="PSUM") as ps, \
             tc.tile_pool(name="dram", bufs=1, space="DRAM") as dram:

            idx_sb = cst.tile([128, SC], i16)
            nc.sync.dma_start(idx_sb[:], idx_in[:, :])
            w1s = cst.tile([128, D], bf16)
            nc.sync.dma_start(w1s[:], w1_in[:, :])
            w2s = cst.tile([128, D], bf16)
            nc.sync.dma_start(w2s[:], w2_in[:, :])
            w3s = cst.tile([128, D_OUT_PAD], bf16)
            nc.sync.dma_start(w3s[:], w3_in[:, :])
            b1s = cst.tile([1, D], bf16)
            nc.sync.dma_start(b1s[:], b1_in[:, :])
            b2s = cst.tile([1, D], bf16)
            nc.sync.dma_start(b2s[:], b2_in[:, :])
            b3s = cst.tile([1, D_OUT_PAD], bf16)
            nc.sync.dma_start(b3s[:], b3_in[:, :])
            invd_sb = cst.tile([1, NTILES * 128], bf16)
            nc.sync.dma_start(invd_sb[:], invd_in[:, :])
            dinv_sb = cst.tile([128, NTILES], f32)
            nc.sync.dma_start(dinv_sb[:], dinv_in[:, :])
            dinv2_sb = cst.tile([128, NTILES], f32)
            nc.sync.dma_start(dinv2_sb[:], dinv2_in[:, :])
            ident = cst.tile([128, 128], bf16)
            nc.sync.dma_start(ident[:], id_in[:, :])
            identF = cst.tile([128, 128], f32)
            nc.vector.tensor_copy(identF[:], ident[:])
            zrow = cst.tile([128, D], bf16)
            nc.vector.memset(zrow[:], 0.0)

            sh1 = dram.tile([SR, D], bf16, tag="sh1")
            sh2 = dram.tile([SR, D], bf16, tag="sh2")
            # chunked tables: contiguous Shared tensors, one AG writer each
            tchunks = []
            for ln in (1, 2):
                cs = [nc.dram_tensor(f"tb{ln}c{k}",
                                     [NCORES * (p[k + 1] - p[k]), D], bf16,
                                     addr_space="Shared")
                      for k in range(NCHUNK)]
                a0 = nc.lookup_mls(cs[0]).memorylocations[0].addr
                for k in range(1, NCHUNK):
                    ak = nc.lookup_mls(cs[k]).memorylocations[0].addr
                    exp = a0 + tbl_base[k] * D * 2
                    assert ak == exp, (ln, k, ak, exp)
                tchunks.append(cs)

            _, chunk_sets = _tile_order()
            chunk_of_tile = {}
            for ci, cs in enumerate(chunk_sets):
                for t in cs:
                    chunk_of_tile[t] = ci
            rg = [list(range(NCORES))]

            def emit_ag(sh, chunks, ck):
                if NO_AG:
                    return
                nc.gpsimd.collective_compute(
                    "AllGather", mybir.AluOpType.bypass,
                    replica_groups=rg,
                    ins=[sh[p[ck]:p[ck + 1], :].opt()],
                    outs=[chunks[ck][:, :].opt()])

            for l in range(NLAYERS):
                fo = D if l < 2 else D_OUT_PAD
                W = (w1s, w2s, w3s)[l]
                br = (b1s, b2s, b3s)[l]
                src = (t0_in[BASE:, :] if l == 0
                       else tchunks[l - 1][BASE_CHUNK][:, :])
                sh = (sh1, sh2, None)[l]
                ch_next = (tchunks[0], tchunks[1], None)[l]
                pend_ag = []
                col = 0
                done = [0] * NCHUNK   # tiles emitted per chunk
                for k, (tlist, nidx) in enumerate(calls):
                    nid = nidx + 128
                    gout = gp.tile([128, GMAX // 128, D], bf16, tag="g")
                    nc.gpsimd.dma_gather(
                        out_ap=gout[:, :nid // 128, :],
                        in_ap=src,
                        idxs_ap=idx_sb[:, col:col + nid // 16],
                        num_idxs=nid, num_idxs_reg=nid, elem_size=D,
                        single_packet=False, queue_num=k % NQ)
                    col += nid // 16
                    while (pend_ag and pend_ag[0][0] <= k
                           and k < len(calls) - 1):
                        emit_ag(sh, ch_next, pend_ag.pop(0)[1])
                    if RDELAY:
                        dly = wrk.tile([128, RDELAY], f32, tag="dly")
                        nc.vector.memset(dly[:], 0.0)
                    off = 0
                    for t in tlist:
                        nbt = int(nb[t])
                        pt = min(128, SHARD - t * 128)
                        pa = ps.tile([128, 128], f32, tag="pa")
                        for b in range(nbt):
                            nc.tensor.matmul(pa[:], lhsT=ident[:],
                                             rhs=gout[:, off + b, :],
                                             start=(b == 0),
                                             stop=(b == nbt - 1))
                        off += nbt
                        aggS = wrk.tile([128, 128], f32, tag="aggS")
                        nc.scalar.activation(aggS[:], pa[:], AF.Copy)
                        paT = ps.tile([128, 128], f32, tag="paT")
                        nc.tensor.transpose(paT[:], aggS[:], identF[:])
                        agg = wrk.tile([128, 128], bf16, tag="agg")
                        nc.scalar.activation(agg[:], paT[:], AF.Copy)
                        pst = ps.tile([128, fo], f32, tag="ps")
                        nc.tensor.matmul(pst[:], lhsT=agg[:], rhs=W[:],
                                         start=True, stop=False)
                        nc.tensor.matmul(
                            pst[:],
                            lhsT=invd_sb[0:1, t * 128:(t + 1) * 128],
                            rhs=br[0:1, :], start=False, stop=True)
                        if l < 2:
                            hsb = wrk.tile([128, D], bf16, tag="hsb")
                            nc.scalar.activation(hsb[:], pst[:], AF.Relu,
                                                 scale=dinv2_sb[:, t:t + 1])
                            nc.sync.dma_start(sh[t * 128:t * 128 + pt, :],
                                              hsb[:pt, :])
                            if t == NTILES - 1:
                                nc.sync.dma_start(sh[SHARD:SR, :],
                                                  zrow[0:SR - SHARD, :])
                            ck = chunk_of_tile[t]
                            done[ck] += 1
                            if done[ck] == CHUNK_TILES[ck]:
                                pend_ag.append((k + 2, ck))
                        else:
                            zt = wrk.tile([128, D_OUT_PAD], f32, tag="zt")
                            nc.scalar.activation(zt[:], pst[:], AF.Copy,
                                                 scale=dinv_sb[:, t:t + 1])
                            mx = wrk.tile([128, 1], f32, tag="mx")
                            nc.vector.tensor_reduce(mx[:], zt[:, :D_OUT],
                                                    axis=mybir.AxisListType.X,
                                                    op=mybir.AluOpType.max)
                            nmx = wrk.tile([128, 1], f32, tag="nmx")
                            nc.vector.tensor_scalar_mul(nmx[:], mx[:], -1.0)
                            ex = wrk.tile([128, D_OUT], f32, tag="ex")
                            se = wrk.tile([128, 1], f32, tag="se")
                            nc.scalar.activation(ex[:], zt[:, :D_OUT],
                                                 AF.Exp, bias=nmx[:, 0:1],
                                                 accum_out=se[:, 0:1])
                            lse = wrk.tile([128, 1], f32, tag="lse")
                            nc.scalar.activation(lse[:], se[:], AF.Ln)
                            ot = wrk.tile([128, D_OUT], f32, tag="ot")
                            nc.vector.tensor_scalar(
                                ot[:], zt[:, :D_OUT],
                                scalar1=mx[:, 0:1], scalar2=lse[:, 0:1],
                                op0=mybir.AluOpType.subtract,
                                op1=mybir.AluOpType.subtract)
                            nc.sync.dma_start(y_out[t * 128:t * 128 + pt, :],
                                              ot[:pt, :])
                # layer-end flush: remaining chunk AGs in completion order
                # (BASE chunk's tiles were processed last, so it flushes last)
                for (_, ck) in pend_ag:
                    emit_ag(sh, ch_next, ck)

            if DBG:
                nc.sync.dma_start(dbg_sh[:, :], sh1[:, :])
                for ck in range(NCHUNK):
                    nc.sync.dma_start(
                        dbg_tb[tbl_base[ck]:tbl_base[ck + 1], :],
                        tchunks[0][ck][:, :])

    nc.compile()
    return nc


def prepare(x, src, dst, W1, b1, W2, b2, W3, b3,
            g1, be1, m1, v1, g2, be2, m2, v2):
    bf = np.float16
    x = np.asarray(x, np.float32)
    src = np.asarray(src, np.int64)
    dst = np.asarray(dst, np.int64)
    (dinv, tid, nb, calls, idx_wrapped, dinv_t, dinv2_t, invd_row,
     shard_nodes, p, tbl_base, zid) = _preprocess(src, dst)
    nc = _build(nb, calls, p, tbl_base)

    s1 = np.asarray(g1, np.float32) / np.sqrt(np.asarray(v1, np.float32)
                                              + BN_EPS)
    s2 = np.asarray(g2, np.float32) / np.sqrt(np.asarray(v2, np.float32)
                                              + BN_EPS)
    w1p = (np.asarray(W1, np.float32) * s1[None, :]).astype(bf)
    w2p = (np.asarray(W2, np.float32) * s2[None, :]).astype(bf)
    b1p = ((np.asarray(b1, np.float32) - np.asarray(m1, np.float32)) * s1
           + np.asarray(be1, np.float32)).astype(bf)[None, :]
    b2p = ((np.asarray(b2, np.float32) - np.asarray(m2, np.float32)) * s2
           + np.asarray(be2, np.float32)).astype(bf)[None, :]
    w3p = np.zeros((128, D_OUT_PAD), np.float32)
    w3p[:, :D_OUT] = np.asarray(W3, np.float32)
    w3p = w3p.astype(bf)
    b3p = np.zeros((1, D_OUT_PAD), np.float32)
    b3p[0, :D_OUT] = np.asarray(b3, np.float32)
    b3p = b3p.astype(bf)

    t0 = np.zeros((TBL, D), np.float32)
    t0[tid] = x * dinv[:, None]
    t0 = t0.astype(bf)

    in_maps = []
    for c in range(NCORES):
        in_maps.append({
            "t0": t0, "idx": idx_wrapped[c],
            "w1": w1p, "w2": w2p, "w3": w3p,
            "b1r": b1p, "b2r": b2p, "b3r": b3p,
            "invd": invd_row[c].astype(bf),
            "ident": np.eye(128, dtype=bf),
            "dinvt": dinv_t[c], "dinv2t": dinv2_t[c],
        })
    return nc, in_maps, shard_nodes


def kernel(**inputs):
    from concourse.bass_utils import run_bass_kernel_spmd

    nc, in_maps, shard_nodes = prepare(**inputs)
    res = run_bass_kernel_spmd(nc, in_maps, core_ids=list(range(NCORES)))
    out = np.zeros((N, D_OUT), np.float32)
    for c in range(NCORES):
        out[shard_nodes[c][:SHARD]] = res.results[c]["y"]
    return out
